# revision 1
# baseline (speedup 1.0000x reference)
"""Trainium2 Bass kernel for nn_AnalyticalStage2.

Math (per batch element b, time index i):
    alpha = E1*E2 / ((E1+E2)*eta)
    A     = C/(E1+E2)
    D     = C*E1/(E2*(E1+E2))
    decay d = exp(-alpha * dt)   (uniform grid -> constant per b)
    s_i = d*s_{i-1} + dp_i,  omega_i = (A+D)*p_i - D*s_i

Reformulation (no dp pass, no cancellation):
    v_i = d*v_{i-1} + p_i          (first-order scan directly on p)
    omega_i = A*p_i + c*v_{i-1},   c = D*(1-d)
The combine is OUTPUT-side, so bf16 quantization of p/v/A/c gives a flat
~2e-3 relative error with no 1/(1-d) amplification (unlike folding the
coefficients into the recurrence input).

Mapping: 512 batch rows -> 8 cores x 64 rows (embarrassingly parallel).
Per core the 32768-step sequence is split in two 16384-step halves on
128 partitions (partition = h*64 + b).  Per W-tile:
  - GpSimd SWDGE DMA loads p as bf16 (casts f32->bf16 in flight),
  - DVE tensor_tensor_scan (fp32 internal state, bf16 output) computes v,
  - TensorEngine combines om = diag(A) @ p + diag(c) @ v_shifted into
    PSUM (bf16 full-rate matmuls, fp32 accumulate),
  - ACT drains PSUM: half-1 rows to a stage tile (DMA'd out), half-2
    rows into om2buf.
Half 2 is scanned with initial state 0 and fixed up in the tail:
    omega2[i] += (c*v1_end) * d^i
via a geometric tile G0[i] = d^i (built by doubling on ACT) and
per-chunk scalars q_j = c*v1_end*d^(1024j), applied by
scalar_tensor_tensor on DVE in widening chunks, then DMA'd out.
"""

import numpy as np

import concourse.bass as bass
import concourse.bacc as bacc
import concourse.mybir as mybir
from concourse.bass_utils import run_bass_kernel_spmd
from concourse.tile import TileContext

_C = 0.206756
B, NT = 512, 32768
NCORES = 8
BLOC = B // NCORES  # 64
DELTA = 0.2 / (NT - 1)  # uniform grid spacing of t = linspace(0, 0.2, NT)

F32 = mybir.dt.float32
BF16 = mybir.dt.bfloat16
ALU = mybir.AluOpType
ACTF = mybir.ActivationFunctionType

TH = NT // 2  # per-half length 16384
W = 2048  # free-axis tile width
NTILES = TH // W  # 8
MM = 512  # matmul free-dim chunk (one PSUM bank)


def build(nc):
    p_ext = nc.declare_dram_parameter("p", [BLOC, NT], F32, isOutput=False)
    hr_ext = nc.declare_dram_parameter("h_raw", [BLOC, 3], F32, isOutput=False)
    out_ext = nc.declare_dram_parameter("out", [BLOC, NT], F32, isOutput=True)

    # (h, b, t) view: partition = h*64 + b, free = time within half
    out_r = out_ext[:].rearrange("b (h t) -> h b t", h=2)

    with TileContext(nc) as tc:
        with (
            tc.tile_pool(name="const", bufs=1) as cpool,
            tc.tile_pool(name="big", bufs=1) as bigpool,
            tc.tile_pool(name="pbf", bufs=5) as bpool,
            tc.tile_pool(name="vsp", bufs=4) as vpool,
            tc.tile_pool(name="om", bufs=3) as opool,
            tc.tile_pool(name="st", bufs=4) as stpool,
            tc.tile_pool(name="ps", bufs=2, space="PSUM") as pspool,
        ):
            # ---- params, computed on all 128 rows directly ----
            hr = cpool.tile([128, 3], F32)
            nc.gpsimd.dma_start(out=hr[0:64, :], in_=hr_ext[:])
            nc.gpsimd.dma_start(out=hr[64:128, :], in_=hr_ext[:])
            E1, E2, eta = hr[:, 0:1], hr[:, 1:2], hr[:, 2:3]

            prm = cpool.tile([128, 16], F32)

            def pc(i):
                return prm[:, i : i + 1]

            s, se, rse, e12 = pc(0), pc(1), pc(2), pc(3)
            alpha, lnd, d, rs = pc(4), pc(5), pc(6), pc(7)
            A, rE2, t2, t3 = pc(8), pc(9), pc(10), pc(11)
            D, omd, c = pc(12), pc(13), pc(14)

            nc.vector.tensor_add(out=s, in0=E1, in1=E2)
            nc.vector.tensor_mul(out=se, in0=s, in1=eta)
            nc.vector.reciprocal(rse, se)
            nc.vector.tensor_mul(out=e12, in0=E1, in1=E2)
            nc.vector.tensor_mul(out=alpha, in0=e12, in1=rse)
            nc.vector.tensor_scalar_mul(lnd, alpha, -DELTA)
            nc.scalar.activation(d, lnd, ACTF.Exp)
            nc.vector.reciprocal(rs, s)
            nc.vector.tensor_scalar_mul(A, rs, _C)
            nc.vector.reciprocal(rE2, E2)
            nc.vector.tensor_mul(out=t2, in0=E1, in1=rE2)
            nc.vector.tensor_mul(out=t3, in0=t2, in1=rs)
            nc.vector.tensor_scalar_mul(D, t3, _C)
            nc.vector.tensor_scalar(omd, d, -1.0, 1.0, ALU.mult, ALU.add)
            nc.vector.tensor_mul(out=c, in0=D, in1=omd)

            # 0/1 identity mask (single gp op, ahead of the p-tile queue)
            I01 = cpool.tile([128, 128], F32)
            one = cpool.tile([128, 1], F32)
            nc.vector.memset(one[:, :], 1.0)
            nc.gpsimd.affine_select(
                out=I01[:],
                in_=one[:, 0:1].broadcast_to([128, 128]),
                pattern=[[1, 128]],
                compare_op=ALU.is_equal,
                fill=0.0,
                base=0,
                channel_multiplier=-1,
            )

            # prefetch the first p tiles before any gp-engine setup work
            pb_tiles = {}
            for k in range(min(4, NTILES)):
                lo = k * W
                pb = bpool.tile([128, W], BF16, tag="pb")
                nc.gpsimd.dma_start(out=pb[0:64, :], in_=p_ext[:, lo : lo + W])
                nc.gpsimd.dma_start(
                    out=pb[64:128, :], in_=p_ext[:, TH + lo : TH + lo + W]
                )
                pb_tiles[k] = pb

            # diag(A), diag(c) in bf16 for full-rate matmul
            diagA = cpool.tile([128, 128], BF16)
            diagc = cpool.tile([128, 128], BF16)
            nc.vector.tensor_scalar_mul(diagA[:], I01[:], A)
            nc.vector.tensor_scalar_mul(diagc[:], I01[:], c)

            # G0[i] = d^i for i in [0, W) by geometric doubling on ACT:
            # G0[:, k:2k] = G0[:, 0:k] * d^k, with d^(2^j) columns from DVE.
            GW = 2 * W  # fixup chunks up to 2*W wide
            G0 = cpool.tile([128, GW], F32)
            ndbl = GW.bit_length() - 1  # GW = 2**ndbl
            dks = cpool.tile([128, ndbl + 2], F32)
            nc.scalar.copy(out=dks[:, 0:1], in_=d)
            for j in range(1, ndbl + 2):
                nc.vector.tensor_mul(
                    out=dks[:, j : j + 1],
                    in0=dks[:, j - 1 : j],
                    in1=dks[:, j - 1 : j],
                )
            nc.vector.memset(G0[:, 0:1], 1.0)
            kk = 1
            for j in range(ndbl):
                nc.scalar.activation(
                    G0[:, kk : 2 * kk],
                    G0[:, 0:kk],
                    ACTF.Copy,
                    scale=dks[:, j : j + 1],
                )
                kk *= 2

            # half-2 partial omegas (bf16), fixed up in the tail
            om2buf = bigpool.tile([128, TH], BF16)
            G0bf = cpool.tile([128, 2 * W], BF16)

            nc.scalar.copy(out=G0bf[:, :], in_=G0[:, :])

            # ---- streaming phase ----
            zcol = cpool.tile([128, 1], BF16)
            nc.vector.memset(zcol[:, :], 0.0)

            prev_vs = None
            for k in range(NTILES):
                lo = k * W
                # p tile, cast to bf16 in flight (SWDGE)
                if k in pb_tiles:
                    pb = pb_tiles[k]
                else:
                    pb = bpool.tile([128, W], BF16, tag="pb")
                    nc.gpsimd.dma_start(out=pb[0:64, :], in_=p_ext[:, lo : lo + W])
                    nc.gpsimd.dma_start(
                        out=pb[64:128, :], in_=p_ext[:, TH + lo : TH + lo + W]
                    )

                # vstripe[:, i+1] = v[lo+i] (bf16 out, fp32 state); [:, 0] = v[lo-1]
                vs = vpool.tile([128, W + 1], BF16)
                init = zcol[:, 0:1] if prev_vs is None else prev_vs[:, W : W + 1]
                nc.vector.tensor_tensor_scan(
                    out=vs[:, 1 : W + 1],
                    data0=d.broadcast_to([128, W]),
                    data1=pb[:],
                    initial=init,
                    op0=ALU.mult,
                    op1=ALU.add,
                )
                nc.scalar.copy(out=vs[:, 0:1], in_=init)

                # om = diag(A) @ p + diag(c) @ v_shifted   (PSUM accumulate)
                ps = pspool.tile([128, W], F32)
                for j in range(W // MM):
                    nc.tensor.matmul(
                        ps[:, j * MM : (j + 1) * MM],
                        diagA[:],
                        pb[:, j * MM : (j + 1) * MM],
                        start=True,
                        stop=False,
                    )
                for j in range(W // MM):
                    nc.tensor.matmul(
                        ps[:, j * MM : (j + 1) * MM],
                        diagc[:],
                        vs[:, j * MM : j * MM + MM],
                        start=False,
                        stop=True,
                    )

                om = opool.tile([128, W], F32)
                nc.scalar.copy(out=om[0:64, :], in_=ps[0:64, :])
                nc.sync.dma_start(out=out_r[0, :, lo : lo + W], in_=om[0:64, :])
                nc.scalar.copy(out=om2buf[64:128, lo : lo + W], in_=ps[64:128, :])
                prev_vs = vs

            # ---- tail: fix up half 2 ----
            # qfree[:, j] = c * v1_end * d^(1024*j)   (partitions 64:128)
            NQ = TH // 1024
            qfree = cpool.tile([128, NQ], F32)
            v1e = cpool.tile([128, 1], F32)
            nc.gpsimd.dma_start(out=v1e[64:128, :], in_=prev_vs[0:64, W : W + 1])
            nc.vector.tensor_mul(
                out=qfree[64:128, 0:1], in0=v1e[64:128, :], in1=prm[64:128, 14:15]
            )
            # doubling: qfree[k:2k] = qfree[0:k] * d^(1024k)
            kq = 1
            while kq < NQ:
                j = 10 + kq.bit_length() - 1  # dks[j] = d^(1024*kq)
                nc.vector.tensor_scalar_mul(
                    qfree[64:128, kq : 2 * kq],
                    qfree[64:128, 0:kq],
                    dks[64:128, j : j + 1],
                )
                kq *= 2

            CHUNKS = [(0, 1024), (1024, 1024), (2048, 2048), (4096, 4096),
                      (8192, 4096), (12288, 4096)]
            for lo, cw in CHUNKS:
                tmp = stpool.tile([128, 2 * W], BF16, tag="tmpbf")
                stage = stpool.tile([128, 2 * W], BF16, tag="stage")
                nc.vector.tensor_scalar_mul(
                    tmp[64:128, 0:cw],
                    G0bf[64:128, 0:cw],
                    qfree[64:128, lo // 1024 : lo // 1024 + 1],
                )
                nc.vector.tensor_add(
                    out=stage[64:128, 0:cw],
                    in0=tmp[64:128, 0:cw],
                    in1=om2buf[64:128, lo : lo + cw],
                )
                nc.gpsimd.dma_start(
                    out=out_r[1, :, lo : lo + cw], in_=stage[64:128, 0:cw]
                )

    return nc


def _shard(x):
    return [np.ascontiguousarray(x[i * BLOC : (i + 1) * BLOC]) for i in range(NCORES)]


def make_nc():
    nc = bacc.Bacc(None)
    build(nc)
    nc.finalize()
    return nc


def run(inputs, trace=False):
    nc = make_nc()
    p_sh = _shard(np.asarray(inputs["p"], dtype=np.float32))
    hr_sh = _shard(np.asarray(inputs["h_raw"], dtype=np.float32))
    in_maps = [{"p": p_sh[i], "h_raw": hr_sh[i]} for i in range(NCORES)]
    res = run_bass_kernel_spmd(nc, in_maps, core_ids=list(range(NCORES)), trace=trace)
    out = np.concatenate([res.results[i]["out"] for i in range(NCORES)], axis=0)
    return out, res


def kernel(h, t, p, h_raw):
    out, _ = run({"p": p, "h_raw": h_raw})
    return out

